# revision 19
# baseline (speedup 1.0000x reference)
"""Trainium2 Bass kernel for nn_Mismatch_loss (weighted per-channel MSE loss).

Contract: kernel(**inputs) takes FULL fp32 inputs (net_out, target,
max_positiones of shape [8, 16, 384, 384]) and returns the FULL scalar
output, distributing work across 8 NeuronCores internally.

Sharding: data-parallel over batch — core b processes image b.

Math per (b, c) channel (spatial reductions over 384*384 = HW elements):
    d   = t - n
    d2  = d * d
    S1  = sum(t)        (= d1 in the reference)
    S2  = sum(d2)       (= m1 + m2)
    S3  = sum(d2 * t)   (= m1)
    loss = ALPHA*S3/(S1+eps) + (1-ALPHA)*(S2-S3)/(HWE-S1+eps)
The tiny [B, C] -> scalar finalization (active-mask, count of nonzero
losses, means) runs on host from the gathered per-channel sums.

Device layout per core: host uploads ONE combined tensor
x_in[128, C*2304] fp16, partition-major, where channel c occupies
columns [c*2304, (c+1)*2304) = [t(1152) | n(1152)].  Every DMA
descriptor is a 4608B contiguous run per partition.  A single HW DMA
queue ring tops out well below the HBM rate, so the input stream is
split across the two hardware-DGE rings: qSyncDynamicHW (SP) carries
the even channels, qActDynamicHW (Activation) carries the odd channels
— measured together they sustain ~415 GB/s.  ACT's eight DMA issues are
interleaved with its squares so the qAct ring stays ~3 channels ahead
of its stream.  (Alternatives measured worse: the Pool/SWDGE ring
starves the sync ring, and any Pool-engine compute halves DVE's
tensor_tensor throughput via SBUF port contention.)

Engines per channel:
  - DVE: d = t - n, p = d2 * t      (fp16 tensor_tensor, 2x mode)
  - ACT: d2 = Square(d) with accum_out -> per-partition sum(d2) column
  - PE : per-channel column sums of t and p via one-hot fp16 weights,
         accumulated across chunks/channels into PSUM [16, 512]
  - DVE: final PSUM -> [16,1] reductions (no ACT table reload)

Inputs are cast to fp16 on host before upload: halves HBM traffic (the
kernel is DMA-bound) at ~1e-5 relative error on the final scalar.

max_positiones is only consulted when a channel of target is exactly
all-zero (cannot happen for this problem's random-uniform inputs); that
case is handled exactly on host without shipping the tensor to devices.
"""

import os
import sys

import numpy as np

for _p in ("/opt/trn_rl_repo", "/root/.axon_site/_ro/trn_rl_repo"):
    if os.path.isdir(_p) and _p not in sys.path:
        sys.path.append(_p)

B, C, H, W = 8, 16, 384, 384
HWE = H * W          # 147456 spatial elements per channel
P = 128              # SBUF partitions
F = HWE // P         # 1152 elements per partition per channel
F2 = 2 * F           # t|n combined row per channel
CHUNKS = (512, 512, 128)   # PE matmul free-dim chunking of F
SMOOTH = 1e-6
ALPHA = 0.05

# Pipeline slots (v7): physical channels 0 and 15 are split into
# half-width slots.  The first halves stream in parallel on the two DMA
# rings, starting ACT's square pipeline ~1.4us earlier; the last halves
# shorten the end-of-stream serial chain (sub->square->mul->matmul) by
# ~1us.  Slot k occupies x_in cols [SLOT_OFF[k], SLOT_OFF[k]+2*SLOT_W[k])
# as [t(W) | n(W)].
SLOT_W = (576,) * 6 + (F,) * 12 + (576, 576)
SLOT_OFF = (
    (0, F, F2, F2 + F, 2 * F2, 2 * F2 + F)
    + tuple(c * F2 for c in range(3, 15))
    + (15 * F2, 15 * F2 + F)
)
NSLOT = len(SLOT_W)          # 20
# slot -> physical channel (for host-side merge)
SLOT_CH = (0, 0, 1, 1, 2, 2) + tuple(range(3, 15)) + (15, 15)

_CACHE = {}


def _build_bass_v2q():
    import concourse.bass as bass
    import concourse.mybir as mybir

    f16 = mybir.dt.float16
    f32 = mybir.dt.float32
    Alu = mybir.AluOpType
    Act = mybir.ActivationFunctionType

    RING = 6                     # d/d2/p ring depth (channels in flight)
    SKEW = 2                     # subs lead muls by SKEW channels

    nc = bass.Bass("TRN2", target_bir_lowering=False, debug=False, num_devices=1)
    x_in = nc.dram_tensor("x_in", [P, C * F2], f16, kind="ExternalInput")
    # Merged output: cols 0..15 = per-partition sum(d2) (acc2);
    # [0:16, 16] = per-channel sum(t); [0:16, 17] = per-channel sum(d2*t).
    out_all = nc.dram_tensor("out_all", [P, C + 2], f32, kind="ExternalOutput")

    from contextlib import ExitStack

    with ExitStack() as ctx:
        ctx.enter_context(nc.cleanup_on_exit())
        sb = lambda name, shape, dtype: ctx.enter_context(  # noqa: E731
            nc.sbuf_tensor(name, shape, dtype)
        )
        x_sb = [sb(f"x_sb{c}", [P, F2], f16) for c in range(C)]
        d_sb = [sb(f"d_sb{k}", [P, F], f16) for k in range(RING)]
        d2_sb = [sb(f"d2_sb{k}", [P, F], f16) for k in range(RING)]
        p_sb = [sb(f"p_sb{k}", [P, F], f16) for k in range(RING)]
        oneh = sb("oneh_sb", [P, C, 16], f16)
        outb = sb("outb_sb", [P, C + 2], f32)
        scratch = sb("scratch_sb", [P, 1], f16)
        psum1 = ctx.enter_context(nc.psum_tensor("psum1", [16, 512], f32))
        psum3 = ctx.enter_context(nc.psum_tensor("psum3", [16, 512], f32))

        sem = nc.alloc_semaphore
        s_x = [sem(f"s_x{c}") for c in range(C)]
        s_oneh = sem("s_oneh")
        s_d = sem("s_d")      # subs completed
        s_sq = sem("s_sq")    # squares completed
        s_p = sem("s_p")      # muls completed
        s_pet = sem("s_pet")  # PE t-matmul channels completed
        s_pep = sem("s_pep")  # PE p-matmul channels completed
        s_red = sem("s_red")  # final reductions completed
        s_out = sem("s_out")  # output DMA completed

        def t_ap(c):
            return x_sb[c][:, 0:F]

        def n_ap(c):
            return x_sb[c][:, F:F2]

        # ---- Input DMAs, split across the two HWDGE queues ----
        def in_dma(eng, c):
            eng.dma_start(
                x_sb[c][:, :], x_in.ap()[:, c * F2 : (c + 1) * F2]
            ).then_inc(s_x[c], 16)

        for c in range(0, C, 2):  # SP: even channels
            in_dma(nc.sync, c)
        # acc2 columns ship as soon as the squares finish (overlaps the
        # final muls/matmuls); the tiny reduction outputs ship last.
        nc.sync.wait_ge(s_sq, C)
        nc.sync.dma_start(
            out_all.ap()[:, 0:C], outb[:, 0:C]
        ).then_inc(s_out, 16)
        nc.sync.wait_ge(s_red, 2)
        nc.sync.dma_start(
            out_all.ap()[0:16, C : C + 2], outb[0:16, C : C + 2]
        ).then_inc(s_out, 16)
        nc.sync.wait_ge(s_out, 32)

        # ---- GPSIMD: build one-hot weights on device (no DMA needed) ----
        nc.gpsimd.memset(oneh[:, :, :], 0.0)
        for c in range(C):
            ms = nc.gpsimd.memset(oneh[:, c, c : c + 1], 1.0)
        ms.then_inc(s_oneh, 1)

        # ---- DVE: subs (SKEW channels ahead) and muls ----
        def emit_sub(c):
            nc.vector.wait_ge(s_x[c], 16)
            nc.vector.tensor_tensor(
                d_sb[c % RING][:, :], t_ap(c), n_ap(c), Alu.subtract
            ).then_inc(s_d, 1)

        def emit_mul(j):
            nc.vector.wait_ge(s_sq, j + 1)
            if j >= RING:
                nc.vector.wait_ge(s_pep, j - (RING - 1))
            nc.vector.tensor_tensor(
                p_sb[j % RING][:, :], d2_sb[j % RING][:, :], t_ap(j), Alu.mult
            ).then_inc(s_p, 1)

        for i in range(C + SKEW):
            if i < C:
                emit_sub(i)
            if i - SKEW >= 0:
                emit_mul(i - SKEW)

        # Final PSUM -> [16,1] reductions on DVE (ACT's Copy would need a
        # second activation-table load).
        nc.vector.wait_ge(s_pet, C)
        nc.vector.tensor_reduce(
            outb[0:16, C : C + 1], psum1[:, :],
            axis=mybir.AxisListType.X, op=Alu.add,
        ).then_inc(s_red, 1)
        nc.vector.wait_ge(s_pep, C)
        nc.vector.tensor_reduce(
            outb[0:16, C + 1 : C + 2], psum3[:, :],
            axis=mybir.AxisListType.X, op=Alu.add,
        ).then_inc(s_red, 1)

        # ---- ACT: odd-channel input DMAs + squares ----
        # The first three odd-channel DMAs issue before the table-load
        # dummy; the rest interleave between squares so the qAct ring
        # stays ~3 channels ahead of its stream without delaying the
        # square pipeline.
        odd = list(range(1, C, 2))
        for c in odd[:3]:
            in_dma(nc.scalar, c)
        # Dummy activation: pulls the one-time ACT_TABLE_LOAD (~1.3us)
        # off the critical path of the first real square.
        nc.scalar.activation(scratch[:, :], scratch[:, :], Act.Square)
        for c in range(C):
            nc.scalar.wait_ge(s_d, c + 1)
            if c >= RING:
                nc.scalar.wait_ge(s_p, c - (RING - 1))
            nc.scalar.activation(
                d2_sb[c % RING][:, :],
                d_sb[c % RING][:, :],
                Act.Square,
                accum_out=outb[:, c : c + 1],
            ).then_inc(s_sq, 1)
            if c < len(odd) - 3:
                in_dma(nc.scalar, odd[c + 3])

        # ---- PE: one-hot column-sum matmuls; t leads p by SKEW+1 ----
        def emit_t_mms(c):
            nc.tensor.wait_ge(s_x[c], 16)
            if c == 0:
                nc.tensor.wait_ge(s_oneh, 1)
            w = oneh[:, c, :]
            off = 0
            for wdt in CHUNKS:
                mm = nc.tensor.matmul(
                    psum1[:, 0:wdt],
                    lhsT=w,
                    rhs=x_sb[c][:, off : off + wdt],
                    start=(c == 0 and off == 0),
                    stop=(c == C - 1 and off + wdt == F),
                    skip_group_check=True,
                )
                off += wdt
            mm.then_inc(s_pet, 1)

        def emit_p_mms(c):
            nc.tensor.wait_ge(s_p, c + 1)
            w = oneh[:, c, :]
            off = 0
            for wdt in CHUNKS:
                mm = nc.tensor.matmul(
                    psum3[:, 0:wdt],
                    lhsT=w,
                    rhs=p_sb[c % RING][:, off : off + wdt],
                    start=(c == 0 and off == 0),
                    stop=(c == C - 1 and off + wdt == F),
                    skip_group_check=True,
                )
                off += wdt
            mm.then_inc(s_pep, 1)

        PE_SKEW = 3
        for i in range(C + PE_SKEW):
            if i < C:
                emit_t_mms(i)
            if i - PE_SKEW >= 0:
                emit_p_mms(i - PE_SKEW)

        nc.all_engine_barrier()

    return nc



def _build_bass_v7():
    """Slot-based pipeline: channels 0 and 15 split into half-slots."""
    import concourse.bass as bass
    import concourse.mybir as mybir

    f16 = mybir.dt.float16
    f32 = mybir.dt.float32
    Alu = mybir.AluOpType
    Act = mybir.ActivationFunctionType

    RING = 6
    SKEW = 2
    S = NSLOT

    def chunks_of(w):
        return (512, 64) if w == 576 else CHUNKS

    nc = bass.Bass("TRN2", target_bir_lowering=False, debug=False, num_devices=1)
    x_in = nc.dram_tensor("x_in", [P, C * F2], f16, kind="ExternalInput")
    # cols 0..S-1 = per-partition sum(d2) per slot; [0:S, S] = per-slot
    # sum(t); [0:S, S+1] = per-slot sum(d2*t).
    out_all = nc.dram_tensor("out_all", [P, S + 2], f32, kind="ExternalOutput")

    from contextlib import ExitStack

    with ExitStack() as ctx:
        ctx.enter_context(nc.cleanup_on_exit())
        sb = lambda name, shape, dtype: ctx.enter_context(  # noqa: E731
            nc.sbuf_tensor(name, shape, dtype)
        )
        x_sb = [sb(f"x_sb{k}", [P, 2 * SLOT_W[k]], f16) for k in range(S)]
        d_sb = [sb(f"d_sb{k}", [P, F], f16) for k in range(RING)]
        d2_sb = [sb(f"d2_sb{k}", [P, F], f16) for k in range(RING)]
        p_sb = [sb(f"p_sb{k}", [P, F], f16) for k in range(RING)]
        oneh = sb("oneh_sb", [P, S, S], f16)
        outb = sb("outb_sb", [P, S + 2], f32)
        scratch = sb("scratch_sb", [P, 1], f16)
        psum1 = ctx.enter_context(nc.psum_tensor("psum1", [S, 512], f32))
        psum3 = ctx.enter_context(nc.psum_tensor("psum3", [S, 512], f32))

        sem = nc.alloc_semaphore
        s_x = [sem(f"s_x{k}") for k in range(S)]
        s_oneh = sem("s_oneh")
        s_d = sem("s_d")
        s_sq = sem("s_sq")
        s_p = sem("s_p")
        s_pet = sem("s_pet")
        s_pep = sem("s_pep")
        s_red = sem("s_red")
        s_out = sem("s_out")

        def t_ap(k):
            return x_sb[k][:, 0 : SLOT_W[k]]

        def n_ap(k):
            return x_sb[k][:, SLOT_W[k] : 2 * SLOT_W[k]]

        # ---- Input DMAs, alternating slots across the two HWDGE rings ----
        def in_dma(eng, k):
            eng.dma_start(
                x_sb[k][:, :],
                x_in.ap()[:, SLOT_OFF[k] : SLOT_OFF[k] + 2 * SLOT_W[k]],
            ).then_inc(s_x[k], 16)

        for k in range(0, S, 2):  # SP: even slots
            in_dma(nc.sync, k)
        nc.sync.wait_ge(s_sq, S)
        nc.sync.dma_start(
            out_all.ap()[:, 0:S], outb[:, 0:S]
        ).then_inc(s_out, 16)
        nc.sync.wait_ge(s_red, 2)
        nc.sync.dma_start(
            out_all.ap()[0:S, S : S + 2], outb[0:S, S : S + 2]
        ).then_inc(s_out, 16)
        nc.sync.wait_ge(s_out, 32)

        # ---- GPSIMD: one-hot weights ----
        nc.gpsimd.memset(oneh[:, :, :], 0.0)
        for k in range(S):
            ms = nc.gpsimd.memset(oneh[:, k, k : k + 1], 1.0)
        ms.then_inc(s_oneh, 1)

        # ---- DVE: subs and muls ----
        def emit_sub(k):
            nc.vector.wait_ge(s_x[k], 16)
            if k >= RING:
                nc.vector.wait_ge(s_sq, k - (RING - 1))
            w = SLOT_W[k]
            nc.vector.tensor_tensor(
                d_sb[k % RING][:, 0:w], t_ap(k), n_ap(k), Alu.subtract
            ).then_inc(s_d, 1)

        def emit_mul(j):
            nc.vector.wait_ge(s_sq, j + 1)
            if j >= RING:
                nc.vector.wait_ge(s_pep, j - (RING - 1))
            w = SLOT_W[j]
            nc.vector.tensor_tensor(
                p_sb[j % RING][:, 0:w], d2_sb[j % RING][:, 0:w], t_ap(j),
                Alu.mult,
            ).then_inc(s_p, 1)

        for i in range(S + SKEW):
            if i < S:
                emit_sub(i)
            if i - SKEW >= 0:
                emit_mul(i - SKEW)

        nc.vector.wait_ge(s_pet, S)
        nc.vector.tensor_reduce(
            outb[0:S, S : S + 1], psum1[:, :],
            axis=mybir.AxisListType.X, op=Alu.add,
        ).then_inc(s_red, 1)
        nc.vector.wait_ge(s_pep, S)
        nc.vector.tensor_reduce(
            outb[0:S, S + 1 : S + 2], psum3[:, :],
            axis=mybir.AxisListType.X, op=Alu.add,
        ).then_inc(s_red, 1)

        # ---- ACT: odd-slot input DMAs + squares ----
        odd = list(range(1, S, 2))
        for k in odd[:3]:
            in_dma(nc.scalar, k)
        nc.scalar.activation(scratch[:, :], scratch[:, :], Act.Square)
        for k in range(S):
            nc.scalar.wait_ge(s_d, k + 1)
            if k >= RING:
                nc.scalar.wait_ge(s_p, k - (RING - 1))
            w = SLOT_W[k]
            nc.scalar.activation(
                d2_sb[k % RING][:, 0:w],
                d_sb[k % RING][:, 0:w],
                Act.Square,
                accum_out=outb[:, k : k + 1],
            ).then_inc(s_sq, 1)
            if k < len(odd) - 3:
                in_dma(nc.scalar, odd[k + 3])

        # ---- PE: one-hot column-sum matmuls ----
        def emit_t_mms(k):
            nc.tensor.wait_ge(s_x[k], 16)
            if k == 0:
                nc.tensor.wait_ge(s_oneh, 1)
            w = oneh[:, k, :]
            off = 0
            for wdt in chunks_of(SLOT_W[k]):
                mm = nc.tensor.matmul(
                    psum1[:, 0:wdt],
                    lhsT=w,
                    rhs=x_sb[k][:, off : off + wdt],
                    start=(k == 0 and off == 0),
                    stop=(k == S - 1 and off + wdt == SLOT_W[k]),
                    skip_group_check=True,
                )
                off += wdt
            mm.then_inc(s_pet, 1)

        def emit_p_mms(k):
            nc.tensor.wait_ge(s_p, k + 1)
            w = oneh[:, k, :]
            off = 0
            for wdt in chunks_of(SLOT_W[k]):
                mm = nc.tensor.matmul(
                    psum3[:, 0:wdt],
                    lhsT=w,
                    rhs=p_sb[k % RING][:, off : off + wdt],
                    start=(k == 0 and off == 0),
                    stop=(k == S - 1 and off + wdt == SLOT_W[k]),
                    skip_group_check=True,
                )
                off += wdt
            mm.then_inc(s_pep, 1)

        PE_SKEW = 3
        for i in range(S + PE_SKEW):
            if i < S:
                emit_t_mms(i)
            if i - PE_SKEW >= 0:
                emit_p_mms(i - PE_SKEW)

        nc.all_engine_barrier()

    return nc


def _get_nc():
    v = os.environ.get("BASS_V", "7")
    key = f"nc_v{v}"
    if key not in _CACHE:
        _CACHE[key] = _build_bass_v7() if v == "7" else _build_bass_v2q()
    return _CACHE[key]


def make_in_maps(target, net_out):
    """Per-core input maps: combined [P, C*F2] fp16 partition-major tiles.

    v7 slot layout: each slot k's region is [t(W) | n(W)] at SLOT_OFF[k];
    channels 0 and 15 are stored as two half-slots each."""
    t16 = np.asarray(target, dtype=np.float16).reshape(B, C, P, F)
    n16 = np.asarray(net_out, dtype=np.float16).reshape(B, C, P, F)
    tt = t16.transpose(0, 2, 1, 3)  # [B, P, C, F]
    nn = n16.transpose(0, 2, 1, 3)
    x = np.empty((B, P, C * F2), dtype=np.float16)
    if os.environ.get("BASS_V", "7") == "7":
        half = {}
        for k in range(NSLOT):
            c, w, off = SLOT_CH[k], SLOT_W[k], SLOT_OFF[k]
            h = 0
            if w != F:
                h = half.get(c, 0)
                half[c] = h + w
            x[:, :, off : off + w] = tt[:, :, c, h : h + w]
            x[:, :, off + w : off + 2 * w] = nn[:, :, c, h : h + w]
    else:
        xv = x.reshape(B, P, C, F2)
        xv[:, :, :, 0:F] = tt
        xv[:, :, :, F:F2] = nn
    return [{"x_in": x[b]} for b in range(B)]


def kernel(net_out, target, max_positiones):
    from concourse import bass_utils

    nc = _get_nc()
    in_maps = make_in_maps(target, net_out)

    # The axon terminal occasionally reports the accelerator unrecoverable
    # on the first touch after a previous process ran a NEFF. The failed
    # attempt triggers recovery terminal-side, but the local PJRT client
    # stays poisoned — tear it down between retries.
    last_err = None
    for _attempt in range(4):
        try:
            res = bass_utils.run_bass_kernel_spmd(
                nc, in_maps, core_ids=list(range(8))
            )
            break
        except Exception as e:  # noqa: BLE001
            last_err = e
            import time as _time

            _time.sleep(3.0)
            try:
                import jax

                jax.clear_caches()
                jax.extend.backend.clear_backends()
            except Exception:  # noqa: BLE001
                pass
            _time.sleep(2.0)
    else:
        raise last_err

    S1 = np.zeros((B, C), np.float64)
    S2 = np.zeros((B, C), np.float64)
    S3 = np.zeros((B, C), np.float64)
    v7 = os.environ.get("BASS_V", "7") == "7"
    for b in range(B):
        out = res.results[b]["out_all"].astype(np.float64)
        if v7:
            S = NSLOT
            for k in range(S):
                c = SLOT_CH[k]
                S1[b, c] += out[k, S]
                S3[b, c] += out[k, S + 1]
                S2[b, c] += out[:, k].sum()
        else:
            S1[b] = out[:16, C]
            S3[b] = out[:16, C + 1]
            S2[b] = out[:, :C].sum(axis=0)

    m1, m2, d1 = S3, S2 - S3, S1
    d2n = float(HWE) - d1
    loss = ALPHA * m1 / (d1 + SMOOTH) + (1.0 - ALPHA) * m2 / (d2n + SMOOTH)

    # active-mask: S1 != 0 implies max(target[b,c]) != 0 for non-negative
    # targets; the S1 == 0 corner is resolved exactly on host.
    active = S1 != 0.0
    for b, c in zip(*np.nonzero(~active)):
        mt = np.max(target[b, c])
        mmp = np.max(max_positiones[b, c])
        active[b, c] = not (mt == 0.0 and mmp == 0.0)

    losses = np.where(active, loss, 0.0)
    count = (losses != 0.0).sum(axis=1).astype(np.float64)
    img_losses = losses.sum(axis=1) / count
    return np.float32(img_losses.mean())
